# revision 29
# baseline (speedup 1.0000x reference)
"""Causal multi-head self-attention (B=2, T=2048, D=1024, H=16) on 8 TRN2
NeuronCores.

Sharding (Megatron-style, hardcoded): core = 4*b + g where b in {0,1} is the
batch and g in {0..3} a group of 4 heads. Each core computes Q/K/V projections
for its head group from x[b], fused causal attention for those 4 heads, and a
partial output projection against its 256-column slice of Wo. The host sums
the 4 partial outputs per batch (the all-reduce after out_proj).

On-device design (v3 — software-pipelined emission):
 - All matmul operands bf16 (PSUM accumulation fp32): halves HBM traffic and
   SBUF footprint, enables fast weight loads; rel err ~4e-3 vs the 2e-2 gate.
 - The engines execute in the order the Tile scheduler fixes at compile time,
   which largely follows emission order — so next-chunk projection and
   prev-chunk out-projection matmul groups are EMITTED inside the attention
   kti loop ("filler units"), one per iteration, where they execute during
   the exp latency that otherwise stalls the in-order PE queue and lets the
   HAM clock throttle.  Disjoint PSUM pools per phase kind keep the units
   independent.
 - Scores computed transposed (S^T[k, q]) for two heads at once on disjoint
   PE row groups; softmax skips max subtraction (scores ~N(0, 0.41)). Causal
   masking: whole k-tiles above the diagonal are skipped; diagonal tiles
   masked after exp with gpsimd.affine_select (fill=0).
 - Row sums via a ones-column appended to V (row 64 of the PV accumulator).
   Reciprocal via the [1,512]->[4,128] DMA-reshape trick; the division folds
   into the PSUM->SBUF eviction multiply that writes the bf16 attention
   output, which the per-q-tile out-projection consumes.
"""

from collections import deque

import numpy as np
import ml_dtypes

import concourse.bass as bass
import concourse.tile as tile
from concourse import bacc, mybir
from concourse.bass_utils import run_bass_kernel_spmd

B, T, D, H, DH = 2, 2048, 1024, 16, 64
HPC = 4  # heads per core
GC = 256  # projection columns per core (HPC * DH)
N_CORES = 8
NCH = 4  # chunks of 512 tokens
F32 = mybir.dt.float32
F32R = mybir.dt.float32r
BF16 = mybir.dt.bfloat16
EXP = mybir.ActivationFunctionType.Exp
BF_NP = ml_dtypes.bfloat16

_CACHE = {}


def _build():
    nc = bacc.Bacc(
        "TRN2", target_bir_lowering=False, debug=False, num_devices=N_CORES
    )
    # Pre-swizzled bf16 inputs (host does the transposes):
    #   xs[p, tc, dt, t] = x[b, tc*512+t, dt*128+p]
    #   wq/wk/wv[p, dt, c] = W[g*256+c, dt*128+p]
    #   wo[p, ct, n] = Wo[n, g*256 + ct*128 + p]
    xs = nc.dram_tensor("xs", [128, NCH, 8, 512], BF16, kind="ExternalInput").ap()
    wqs = nc.dram_tensor("wqs", [128, 8, GC], BF16, kind="ExternalInput").ap()
    wks = nc.dram_tensor("wks", [128, 8, GC], BF16, kind="ExternalInput").ap()
    wvs = nc.dram_tensor("wvs", [128, 8, GC], BF16, kind="ExternalInput").ap()
    wos = nc.dram_tensor("wos", [128, 2, D], BF16, kind="ExternalInput").ap()
    out = nc.dram_tensor("out", [T, D], BF16, kind="ExternalOutput").ap()

    with tile.TileContext(nc) as tc:
        with (
            tc.tile_pool(name="persist", bufs=1) as persist,
            tc.tile_pool(name="xtp", bufs=3) as xtp,
            tc.tile_pool(name="qtp", bufs=2) as qtp,
            tc.tile_pool(name="atp", bufs=2) as atp,
            tc.tile_pool(name="ptp", bufs=3) as ptp,
            tc.tile_pool(name="normp", bufs=3) as normp,
            tc.tile_pool(name="outp", bufs=2) as outp,
            tc.tile_pool(name="ps1", bufs=2, space="PSUM") as ps1,
            tc.tile_pool(name="stp", bufs=2, space="PSUM") as stp,
            tc.tile_pool(name="pvp", bufs=2, space="PSUM") as pvp,
        ):
            wq = persist.tile([128, 8, GC], BF16, tag="wq")
            wk = persist.tile([128, 8, GC], BF16, tag="wk")
            wv = persist.tile([128, 8, GC], BF16, tag="wv")
            wo = persist.tile([128, 2, D], BF16, tag="wo")
            # per-chunk K / V stay live for the rest of the kernel
            kt_c = [
                persist.tile([128, 2, 512], BF16, tag=f"kt{c}", name=f"kt{c}")
                for c in range(NCH)
            ]
            vp_c = [
                persist.tile(
                    [128, 4, HPC, DH + 1], BF16, tag=f"vp{c}", name=f"vp{c}"
                )
                for c in range(NCH)
            ]

            ones_sb = persist.tile([128, 64], F32, tag="ones_sb")
            nc.vector.memset(ones_sb[:], 1.0)
            # f32r ones row for the K=1 broadcast matmul in the normalize
            ones1 = persist.tile([1, 64], F32R, tag="ones1")
            nc.vector.tensor_copy(ones1[:], ones_sb[0:1, :])

            xtiles = {}

            def emit_x_dma(c):
                # 2-di pieces: fewer dma_start issues (the ~0.6us SWDGE
                # issue cost is the head bottleneck), still fine-grained
                # enough for compute to chase the stream.
                for j in range(4):
                    xt = xtp.tile(
                        [128, 2, 512], BF16, tag=f"x{j}", name=f"x{c}_{j}"
                    )
                    nc.sync.dma_start(xt[:], xs[:, c, 2 * j : 2 * j + 2])
                    xtiles[(c, j)] = xt

            def xap(c, di):
                return xtiles[(c, di // 2)][:, di % 2, :]

            # Input DMA order: first what phase 1 needs first.
            nc.sync.dma_start(wq[:], wqs[:])
            emit_x_dma(0)
            nc.sync.dma_start(wk[:], wks[:])
            nc.sync.dma_start(wv[:], wvs[:])
            emit_x_dma(1)
            nc.sync.dma_start(wo[:], wos[:])

            qt_c = {}
            at_c = {}

            # ---- phase-1 units (one PSUM accumulation group each) ----
            def qk_unit(c, w_sb, dst, ct):
                def unit():
                    ps = ps1.tile([128, 512], F32, tag="ps", name=f"ps_qk{c}_{ct}")
                    for di in range(8):
                        nc.tensor.matmul(
                            ps[:],
                            w_sb[:, di, ct * 128 : (ct + 1) * 128],
                            xap(c, di),
                            start=(di == 0),
                            stop=(di == 7),
                        )
                    nc.vector.tensor_copy(dst[:, ct, :], ps[:])

                return unit

            def v_unit(c, tt):
                def unit():
                    vp = vp_c[c]
                    if tt == 0:
                        nc.vector.tensor_copy(
                            vp[:, :, :, DH],
                            ones_sb[:, 0:16].rearrange("p (a b) -> p a b", a=4),
                        )
                    ps = ps1.tile([128, 512], F32, tag="ps", name=f"ps_v{c}_{tt}")
                    for di in range(8):
                        nc.tensor.matmul(
                            ps[:, 0:GC],
                            xap(c, di)[:, tt * 128 : (tt + 1) * 128],
                            wv[:, di, :],
                            start=(di == 0),
                            stop=(di == 7),
                        )
                    nc.vector.tensor_copy(
                        vp[:, tt, :, 0:DH],
                        ps[:, 0:GC].rearrange("p (h d) -> p h d", h=HPC),
                    )

                return unit

            def q_units(c):
                qt = qtp.tile([128, 2, 512], BF16, tag="qt", name=f"qt{c}")
                qt_c[c] = qt
                return [qk_unit(c, wq, qt, ct) for ct in range(2)]

            def kv_units(c):
                us = [qk_unit(c, wk, kt_c[c], ct) for ct in range(2)]
                us += [v_unit(c, tt) for tt in range(4)]
                return us

            # ---- phase-3 units (one q-tile of the out-projection each) ----
            def p3_unit(qc, tt):
                def unit():
                    at = at_c[qc]
                    qti = qc * 4 + tt
                    ot = outp.tile([128, 2, 512], BF16, tag="ot", name=f"ot{qti}")
                    for nn in range(2):
                        po3 = ps1.tile(
                            [128, 512], F32, tag="ps", name=f"ps_o{qti}_{nn}"
                        )
                        for ctt in range(2):
                            nc.tensor.matmul(
                                po3[:],
                                at[:, ctt, tt * 128 : (tt + 1) * 128],
                                wo[:, ctt, nn * 512 : (nn + 1) * 512],
                                start=(ctt == 0),
                                stop=(ctt == 1),
                            )
                        nc.vector.tensor_copy(ot[:, nn, :], po3[:])
                    nc.sync.dma_start(
                        out[qti * 128 : (qti + 1) * 128, :].rearrange(
                            "q (a n) -> q a n", a=2
                        ),
                        ot[:],
                    )

                return unit

            def p3_units(qc):
                return [p3_unit(qc, tt) for tt in range(4)]

            # ---- attention loop per chunk, with filler interleaving ----
            def norm_rest(qc, hp, at, stg):
                # the reciprocal + broadcast + divide part of the softmax
                # normalization. Zero DMAs and zero GpSimd: a tiny transfer
                # would queue behind megabytes of bulk x/out traffic in the
                # shared DMA hardware, and GpSimd ops would queue ahead of
                # the diagonal-tile affine_selects (in-order queues). The
                # per-q broadcast is a K=1 outer-product matmul with a ones
                # row; the division folds into the at-write multiply.
                def unit():
                    ct = hp
                    for hh in range(2):
                        po = 64 * hh
                        rr = normp.tile(
                            [1, 512], F32R, tag="rr", name=f"rr{qc}_{hp}{hh}"
                        )
                        with nc.allow_low_precision(reason="f32r out is f32-width"):
                            nc.vector.reciprocal(rr[:], stg[DH : DH + 1, hh, :])
                        rb = ps1.tile(
                            [128, 512], F32, tag="ps", name=f"rb{qc}_{hp}{hh}"
                        )
                        nc.tensor.matmul(
                            rb[0:64, :], ones1[:], rr[:], start=True, stop=True
                        )
                        nc.vector.tensor_mul(
                            at[po : po + 64, ct, :], stg[0:DH, hh, :], rb[0:64, :]
                        )

                return unit

            def phase2(qc, fillers):
                q0 = qc * 512
                at = atp.tile([128, 2, 512], BF16, tag="at", name=f"at{qc}")
                at_c[qc] = at
                qt = qt_c[qc]
                n_kt = 4 * (qc + 1)
                pending = []
                for hp in range(2):  # head pair (2hp, 2hp+1); both have ct = hp
                    ct = hp
                    o0 = pvp.tile([128, 512], F32, tag="pv", name=f"pv{qc}_{hp}0")
                    o1 = pvp.tile([128, 512], F32, tag="pv", name=f"pv{qc}_{hp}1")
                    oo = [o0, o1]
                    for kti in range(n_kt):
                        kb, kw = divmod(kti, 4)
                        st = stp.tile(
                            [128, 2, 512], F32, tag="st", name=f"st{qc}_{hp}_{kti}"
                        )
                        # diagonal tiles: columns q < off are fully masked —
                        # skip them in scores/exp/PV entirely; only the
                        # [off, off+128) window needs the triangular mask.
                        off = max(0, kti * 128 - q0)
                        for hh in range(2):
                            po = 64 * hh
                            # K=64 pair: disjoint PE row groups run concurrently
                            nc.tensor.matmul(
                                st[:, hh, off:],
                                kt_c[kb][
                                    po : po + 64, ct, kw * 128 : (kw + 1) * 128
                                ],
                                qt[po : po + 64, ct, off:],
                                start=True,
                                stop=True,
                            )
                        ptile = ptp.tile(
                            [128, 2, 512], BF16, tag="pt", name=f"pt{qc}_{hp}_{kti}"
                        )
                        nc.scalar.activation(
                            ptile[:, :, off:], st[:, :, off:], EXP, scale=0.125
                        )
                        if kti >= 4 * qc:
                            # valid iff q - k >= 0; q = off + y, k = x
                            nc.gpsimd.affine_select(
                                out=ptile[:, :, off : off + 128],
                                in_=ptile[:, :, off : off + 128],
                                compare_op=mybir.AluOpType.is_ge,
                                fill=0.0,
                                base=0,
                                pattern=[[0, 2], [1, 128]],
                                channel_multiplier=-1,
                            )
                        # filler units execute on the PE / DVE / GpSimd while
                        # exp runs; front-loaded (one per iteration)
                        if fillers:
                            fillers.popleft()()
                        for hh in range(2):
                            nc.tensor.matmul(
                                oo[hh][0 : DH + 1, off:],
                                vp_c[kb][:, kw, 2 * hp + hh, :],
                                ptile[:, hh, off:],
                                start=(kti == 0),
                                stop=(kti == n_kt - 1),
                                skip_group_check=True,
                            )
                    # evict the PV accumulators right away (frees the PSUM
                    # banks for the next head pair / chunk)
                    stg = normp.tile(
                        [DH + 1, 2, 512], F32, tag="stg", name=f"stg{qc}_{hp}"
                    )
                    for hh in range(2):
                        nc.vector.tensor_copy(stg[:, hh, :], oo[hh][0 : DH + 1, :])
                    nr = norm_rest(qc, hp, at, stg)
                    if hp == 0 and qc >= 1:
                        # runs inside hp=1's first (non-diagonal) iteration
                        fillers.appendleft(nr)
                    else:
                        # chunk 0 hp0: hp1's iterations are all diagonal —
                        # emitting the GpSimd chain now would queue it ahead
                        # of their affine_selects. Defer past the loop.
                        pending.append(nr)
                while fillers:
                    fillers.popleft()()
                for nr in pending:
                    nr()

            # ---- emission schedule ----
            qs = {c: q_units(c) for c in range(NCH)}
            for u in qs[0] + kv_units(0):
                u()
            for qc in range(NCH):
                fillers = deque()
                if qc + 1 < NCH:
                    fillers.extend(qs[qc + 1])
                    if qc + 1 < NCH - 1:
                        fillers.extend(kv_units(qc + 1))
                else:
                    # K/V of the last chunk are deferred into its own
                    # (ACT-bound) attention loop; they are only needed by
                    # the diagonal k-tiles at the end of each head pair.
                    fillers.extend(kv_units(qc))
                if qc >= 1:
                    fillers.extend(p3_units(qc - 1))
                if qc + 2 < NCH:
                    emit_x_dma(qc + 2)
                phase2(qc, fillers)
            for u in p3_units(NCH - 1):
                u()
    nc.compile()
    return nc


def _get_nc():
    if "nc" not in _CACHE:
        _CACHE["nc"] = _build()
    return _CACHE["nc"]


def _in_maps(x, Wq, Wk, Wv, Wo):
    x = np.asarray(x, dtype=np.float32).astype(BF_NP)
    Wq = np.asarray(Wq, dtype=np.float32).astype(BF_NP)
    Wk = np.asarray(Wk, dtype=np.float32).astype(BF_NP)
    Wv = np.asarray(Wv, dtype=np.float32).astype(BF_NP)
    Wo = np.asarray(Wo, dtype=np.float32).astype(BF_NP)
    maps = []
    for core in range(N_CORES):
        b, g = divmod(core, 4)
        sl = slice(g * GC, (g + 1) * GC)
        # xs[p, tc, dt, t] = x[b, tc*512+t, dt*128+p]
        xsw = np.ascontiguousarray(
            x[b].reshape(4, 512, 8, 128).transpose(3, 0, 2, 1)
        )
        # w[p, dt, c] = W[sl][c, dt*128+p]
        wqw = np.ascontiguousarray(Wq[sl].reshape(GC, 8, 128).transpose(2, 1, 0))
        wkw = np.ascontiguousarray(Wk[sl].reshape(GC, 8, 128).transpose(2, 1, 0))
        wvw = np.ascontiguousarray(Wv[sl].reshape(GC, 8, 128).transpose(2, 1, 0))
        # wo[p, ct, n] = Wo[n, g*256 + ct*128 + p]
        wow = np.ascontiguousarray(Wo[:, sl].reshape(D, 2, 128).transpose(2, 1, 0))
        maps.append(
            {
                "xs": xsw,
                "wqs": wqw,
                "wks": wkw,
                "wvs": wvw,
                "wos": wow,
            }
        )
    return maps


def _run(x, Wq, Wk, Wv, Wo, **spmd_kwargs):
    nc = _get_nc()
    res = run_bass_kernel_spmd(
        nc, _in_maps(x, Wq, Wk, Wv, Wo), core_ids=list(range(N_CORES)), **spmd_kwargs
    )
    outs = [np.asarray(r["out"]).astype(np.float32) for r in res.results]
    full = np.stack(
        [
            outs[0] + outs[1] + outs[2] + outs[3],
            outs[4] + outs[5] + outs[6] + outs[7],
        ]
    ).astype(np.float32)
    return full, res


def kernel(x, Wq, Wk, Wv, Wo):
    full, _ = _run(x, Wq, Wk, Wv, Wo)
    return full


# revision 43
# speedup vs baseline: 1.2819x; 1.2819x over previous
"""Causal multi-head self-attention (B=2, T=2048, D=1024, H=16) on 8 TRN2
NeuronCores.

Sharding (Megatron-style, hardcoded): core = 4*b + g where b in {0,1} is the
batch and g in {0..3} a group of 4 heads. Each core computes Q/K/V projections
for its head group from x[b], fused causal attention for those 4 heads, and a
partial output projection against its 256-column slice of Wo. The host sums
the 4 partial outputs per batch (the all-reduce after out_proj).

On-device design (v3 — software-pipelined emission):
 - All matmul operands bf16 (PSUM accumulation fp32): halves HBM traffic and
   SBUF footprint, enables fast weight loads; rel err ~4e-3 vs the 2e-2 gate.
 - The engines execute in the order the Tile scheduler fixes at compile time,
   which largely follows emission order — so next-chunk projection and
   prev-chunk out-projection matmul groups are EMITTED inside the attention
   kti loop ("filler units"), one per iteration, where they execute during
   the exp latency that otherwise stalls the in-order PE queue and lets the
   HAM clock throttle.  Disjoint PSUM pools per phase kind keep the units
   independent.
 - Scores computed transposed (S^T[k, q]) for two heads at once on disjoint
   PE row groups; softmax skips max subtraction (scores ~N(0, 0.41)). Causal
   masking: whole k-tiles above the diagonal are skipped; diagonal tiles
   masked after exp with gpsimd.affine_select (fill=0).
 - Row sums via a ones-column appended to V (row 64 of the PV accumulator).
   Reciprocal via the [1,512]->[4,128] DMA-reshape trick; the division folds
   into the PSUM->SBUF eviction multiply that writes the bf16 attention
   output, which the per-q-tile out-projection consumes.
"""

from collections import deque

import numpy as np
import ml_dtypes

import concourse.bass as bass
import concourse.tile as tile
from concourse import bacc, mybir
from concourse.bass_utils import run_bass_kernel_spmd

B, T, D, H, DH = 2, 2048, 1024, 16, 64
HPC = 4  # heads per core
GC = 256  # projection columns per core (HPC * DH)
N_CORES = 8
NCH = 4  # chunks of 512 tokens
F32 = mybir.dt.float32
F32R = mybir.dt.float32r
BF16 = mybir.dt.bfloat16
EXP = mybir.ActivationFunctionType.Exp
BF_NP = ml_dtypes.bfloat16

_CACHE = {}


def _build():
    nc = bacc.Bacc(
        "TRN2", target_bir_lowering=False, debug=False, num_devices=N_CORES
    )
    # Pre-swizzled bf16 inputs (host does the transposes):
    #   xs[p, tc, dt, t] = x[b, tc*512+t, dt*128+p]
    #   wq/wk/wv[p, dt, c] = W[g*256+c, dt*128+p]
    #   wo[p, ct, n] = Wo[n, g*256 + ct*128 + p]
    xs = nc.dram_tensor("xs", [128, NCH, 8, 512], BF16, kind="ExternalInput").ap()
    wqs = nc.dram_tensor("wqs", [128, 8, GC], BF16, kind="ExternalInput").ap()
    wks = nc.dram_tensor("wks", [128, 8, GC], BF16, kind="ExternalInput").ap()
    wvs = nc.dram_tensor("wvs", [128, 8, GC], BF16, kind="ExternalInput").ap()
    wos = nc.dram_tensor("wos", [128, 2, D], BF16, kind="ExternalInput").ap()
    out = nc.dram_tensor("out", [T, D], BF16, kind="ExternalOutput").ap()

    with tile.TileContext(nc) as tc:
        with (
            tc.tile_pool(name="persist", bufs=1) as persist,
            tc.tile_pool(name="xtp", bufs=3) as xtp,
            tc.tile_pool(name="qtp", bufs=2) as qtp,
            tc.tile_pool(name="atp", bufs=2) as atp,
            tc.tile_pool(name="ptp", bufs=3) as ptp,
            tc.tile_pool(name="normp", bufs=8) as normp,
            tc.tile_pool(name="outp", bufs=2) as outp,
            tc.tile_pool(name="ps1", bufs=2, space="PSUM") as ps1,
            tc.tile_pool(name="stp", bufs=2, space="PSUM") as stp,
            tc.tile_pool(name="pvp", bufs=2, space="PSUM") as pvp,
        ):
            wq = persist.tile([128, 8, GC], BF16, tag="wq")
            wk = persist.tile([128, 8, GC], BF16, tag="wk")
            wv = persist.tile([128, 8, GC], BF16, tag="wv")
            wo = persist.tile([128, 2, D], BF16, tag="wo")
            # per-chunk K / V stay live for the rest of the kernel
            kt_c = [
                persist.tile([128, 2, 512], BF16, tag=f"kt{c}", name=f"kt{c}")
                for c in range(NCH)
            ]
            vp_c = [
                persist.tile(
                    [128, 4, HPC, DH + 1], BF16, tag=f"vp{c}", name=f"vp{c}"
                )
                for c in range(NCH)
            ]

            ones_sb = persist.tile([128, 16], F32, tag="ones_sb")
            nc.vector.memset(ones_sb[:], 1.0)

            xtiles = {}

            def emit_x_dma(c):
                # 2-di pieces: fewer dma_start issues (the ~0.6us SWDGE
                # issue cost is the head bottleneck), still fine-grained
                # enough for compute to chase the stream.
                for j in range(4):
                    xt = xtp.tile(
                        [128, 2, 512], BF16, tag=f"x{j}", name=f"x{c}_{j}"
                    )
                    nc.sync.dma_start(xt[:], xs[:, c, 2 * j : 2 * j + 2])
                    xtiles[(c, j)] = xt

            def xap(c, di):
                return xtiles[(c, di // 2)][:, di % 2, :]

            # Input DMA order: first what phase 1 needs first.
            nc.sync.dma_start(wq[:], wqs[:])
            emit_x_dma(0)
            nc.sync.dma_start(wk[:], wks[:])
            nc.sync.dma_start(wv[:], wvs[:])
            emit_x_dma(1)
            nc.sync.dma_start(wo[:], wos[:])

            qt_c = {}
            at_c = {}

            # ---- phase-1 units (one PSUM accumulation group each) ----
            def qk_unit(c, w_sb, dst, ct):
                def unit():
                    ps = ps1.tile([128, 512], F32, tag="ps", name=f"ps_qk{c}_{ct}")
                    for di in range(8):
                        nc.tensor.matmul(
                            ps[:],
                            w_sb[:, di, ct * 128 : (ct + 1) * 128],
                            xap(c, di),
                            start=(di == 0),
                            stop=(di == 7),
                        )
                    nc.vector.tensor_copy(dst[:, ct, :], ps[:])

                return unit

            def v_unit(c, tt):
                def unit():
                    vp = vp_c[c]
                    if tt == 0:
                        nc.vector.tensor_copy(
                            vp[:, :, :, DH],
                            ones_sb[:].rearrange("p (a b) -> p a b", a=4),
                        )
                    ps = ps1.tile([128, 512], F32, tag="ps", name=f"ps_v{c}_{tt}")
                    for di in range(8):
                        nc.tensor.matmul(
                            ps[:, 0:GC],
                            xap(c, di)[:, tt * 128 : (tt + 1) * 128],
                            wv[:, di, :],
                            start=(di == 0),
                            stop=(di == 7),
                        )
                    nc.vector.tensor_copy(
                        vp[:, tt, :, 0:DH],
                        ps[:, 0:GC].rearrange("p (h d) -> p h d", h=HPC),
                    )

                return unit

            def q_units(c):
                qt = qtp.tile([128, 2, 512], BF16, tag="qt", name=f"qt{c}")
                qt_c[c] = qt
                return [qk_unit(c, wq, qt, ct) for ct in range(2)]

            def kv_units(c):
                us = [qk_unit(c, wk, kt_c[c], ct) for ct in range(2)]
                us += [v_unit(c, tt) for tt in range(4)]
                return us

            # ---- phase-3 units (one q-tile of the out-projection each) ----
            def p3_unit(qc, tt):
                def unit():
                    at = at_c[qc]
                    qti = qc * 4 + tt
                    ot = outp.tile([128, 2, 512], BF16, tag="ot", name=f"ot{qti}")
                    for nn in range(2):
                        po3 = ps1.tile(
                            [128, 512], F32, tag="ps", name=f"ps_o{qti}_{nn}"
                        )
                        for ctt in range(2):
                            nc.tensor.matmul(
                                po3[:],
                                at[:, ctt, tt * 128 : (tt + 1) * 128],
                                wo[:, ctt, nn * 512 : (nn + 1) * 512],
                                start=(ctt == 0),
                                stop=(ctt == 1),
                            )
                        nc.vector.tensor_copy(ot[:, nn, :], po3[:])
                    nc.sync.dma_start(
                        out[qti * 128 : (qti + 1) * 128, :].rearrange(
                            "q (a n) -> q a n", a=2
                        ),
                        ot[:],
                    )

                return unit

            def p3_units(qc):
                return [p3_unit(qc, tt) for tt in range(4)]

            # ---- attention loop per chunk, with filler interleaving ----
            def norm_rest(qc, hp, at, stg):
                # the reciprocal + broadcast + divide part of the softmax
                # normalization. Zero DMAs and zero GpSimd: a tiny transfer
                # would queue behind megabytes of bulk x/out traffic in the
                # shared DMA hardware, and GpSimd ops would queue ahead of
                # the diagonal-tile affine_selects (in-order queues). The
                # per-q broadcast is a K=1 outer-product matmul with a ones
                # row; the division folds into the at-write multiply.
                def unit():
                    ct = hp
                    # sums sit on stg row 0 as [1, 2, 512]; reshape them over
                    # 8 lanes via a tiny SBUF->SBUF DMA (rides its own HW
                    # queue), multi-pass reciprocal (free-size bound), DMA
                    # back, broadcast over 64 partitions, and fold the divide
                    # into the bf16 at-write.
                    s8 = normp.tile([8, 128], F32, tag="s8", name=f"s8_{qc}{hp}")
                    nc.sync.dma_start(s8[:], stg[DH : DH + 1, :, :])
                    nc.vector.reciprocal(s8[:], s8[:])
                    rr = normp.tile([1, 2, 512], F32, tag="rr", name=f"rr{qc}{hp}")
                    nc.sync.dma_start(rr[:], s8[:])
                    for hh in range(2):
                        po = 64 * hh
                        rb = normp.tile(
                            [64, 512], F32, tag="rb", name=f"rb{qc}_{hp}{hh}"
                        )
                        nc.gpsimd.partition_broadcast(rb[:], rr[:, hh, :])
                        nc.vector.tensor_mul(
                            at[po : po + 64, ct, :],
                            stg[0:DH, hh, :],
                            rb[:],
                        )

                return unit

            def phase2(qc, fillers):
                q0 = qc * 512
                at = atp.tile([128, 2, 512], BF16, tag="at", name=f"at{qc}")
                at_c[qc] = at
                qt = qt_c[qc]
                n_kt = 4 * (qc + 1)
                pending = []
                for hp in range(2):  # head pair (2hp, 2hp+1); both have ct = hp
                    ct = hp
                    o0 = pvp.tile([128, 512], F32, tag="pv", name=f"pv{qc}_{hp}0")
                    o1 = pvp.tile([128, 512], F32, tag="pv", name=f"pv{qc}_{hp}1")
                    oo = [o0, o1]
                    for kti in range(n_kt):
                        kb, kw = divmod(kti, 4)
                        st = stp.tile(
                            [128, 2, 512], F32, tag="st", name=f"st{qc}_{hp}_{kti}"
                        )
                        # diagonal tiles: columns q < off are fully masked —
                        # skip them in scores/exp/PV entirely; only the
                        # [off, off+128) window needs the triangular mask.
                        off = max(0, kti * 128 - q0)
                        for hh in range(2):
                            po = 64 * hh
                            # K=64 pair: disjoint PE row groups run concurrently
                            nc.tensor.matmul(
                                st[:, hh, off:],
                                kt_c[kb][
                                    po : po + 64, ct, kw * 128 : (kw + 1) * 128
                                ],
                                qt[po : po + 64, ct, off:],
                                start=True,
                                stop=True,
                            )
                        ptile = ptp.tile(
                            [128, 2, 512], BF16, tag="pt", name=f"pt{qc}_{hp}_{kti}"
                        )
                        nc.scalar.activation(
                            ptile[:, :, off:], st[:, :, off:], EXP, scale=0.125
                        )
                        if kti >= 4 * qc:
                            # valid iff q - k >= 0; q = off + y, k = x
                            nc.gpsimd.affine_select(
                                out=ptile[:, :, off : off + 128],
                                in_=ptile[:, :, off : off + 128],
                                compare_op=mybir.AluOpType.is_ge,
                                fill=0.0,
                                base=0,
                                pattern=[[0, 2], [1, 128]],
                                channel_multiplier=-1,
                            )
                        # filler units execute on the PE / DVE / GpSimd while
                        # exp runs; front-loaded (one per iteration)
                        if fillers:
                            fillers.popleft()()
                        for hh in range(2):
                            nc.tensor.matmul(
                                oo[hh][0 : DH + 1, off:],
                                vp_c[kb][:, kw, 2 * hp + hh, :],
                                ptile[:, hh, off:],
                                start=(kti == 0),
                                stop=(kti == n_kt - 1),
                                skip_group_check=True,
                            )
                    # evict the PV accumulators right away (frees the PSUM
                    # banks for the next head pair / chunk)
                    stg = normp.tile(
                        [DH + 1, 2, 512], F32, tag="stg", name=f"stg{qc}_{hp}"
                    )
                    for hh in range(2):
                        nc.vector.tensor_copy(stg[:, hh, :], oo[hh][0 : DH + 1, :])
                    nr = norm_rest(qc, hp, at, stg)
                    if hp == 0 and qc >= 1:
                        # runs inside hp=1's first (non-diagonal) iteration
                        fillers.appendleft(nr)
                    else:
                        # chunk 0 hp0: hp1's iterations are all diagonal —
                        # emitting the GpSimd chain now would queue it ahead
                        # of their affine_selects. Defer past the loop.
                        pending.append(nr)
                while fillers:
                    fillers.popleft()()
                for nr in pending:
                    nr()

            # ---- emission schedule ----
            qs = {c: q_units(c) for c in range(NCH)}
            for u in qs[0] + kv_units(0):
                u()
            for qc in range(NCH):
                fillers = deque()
                if qc + 1 < NCH:
                    fillers.extend(qs[qc + 1])
                    if qc + 1 < NCH - 1:
                        fillers.extend(kv_units(qc + 1))
                else:
                    # K/V of the last chunk are deferred into its own
                    # (ACT-bound) attention loop; they are only needed by
                    # the diagonal k-tiles at the end of each head pair.
                    fillers.extend(kv_units(qc))
                if qc >= 1:
                    fillers.extend(p3_units(qc - 1))
                if qc + 2 < NCH:
                    emit_x_dma(qc + 2)
                phase2(qc, fillers)
            for u in p3_units(NCH - 1):
                u()
    nc.compile()
    return nc


def _get_nc():
    if "nc" not in _CACHE:
        _CACHE["nc"] = _build()
    return _CACHE["nc"]


def _in_maps(x, Wq, Wk, Wv, Wo):
    x = np.asarray(x, dtype=np.float32).astype(BF_NP)
    Wq = np.asarray(Wq, dtype=np.float32).astype(BF_NP)
    Wk = np.asarray(Wk, dtype=np.float32).astype(BF_NP)
    Wv = np.asarray(Wv, dtype=np.float32).astype(BF_NP)
    Wo = np.asarray(Wo, dtype=np.float32).astype(BF_NP)
    maps = []
    for core in range(N_CORES):
        b, g = divmod(core, 4)
        sl = slice(g * GC, (g + 1) * GC)
        # xs[p, tc, dt, t] = x[b, tc*512+t, dt*128+p]
        xsw = np.ascontiguousarray(
            x[b].reshape(4, 512, 8, 128).transpose(3, 0, 2, 1)
        )
        # w[p, dt, c] = W[sl][c, dt*128+p]
        wqw = np.ascontiguousarray(Wq[sl].reshape(GC, 8, 128).transpose(2, 1, 0))
        wkw = np.ascontiguousarray(Wk[sl].reshape(GC, 8, 128).transpose(2, 1, 0))
        wvw = np.ascontiguousarray(Wv[sl].reshape(GC, 8, 128).transpose(2, 1, 0))
        # wo[p, ct, n] = Wo[n, g*256 + ct*128 + p]
        wow = np.ascontiguousarray(Wo[:, sl].reshape(D, 2, 128).transpose(2, 1, 0))
        maps.append(
            {
                "xs": xsw,
                "wqs": wqw,
                "wks": wkw,
                "wvs": wvw,
                "wos": wow,
            }
        )
    return maps


def _run(x, Wq, Wk, Wv, Wo, **spmd_kwargs):
    nc = _get_nc()
    res = run_bass_kernel_spmd(
        nc, _in_maps(x, Wq, Wk, Wv, Wo), core_ids=list(range(N_CORES)), **spmd_kwargs
    )
    outs = [np.asarray(r["out"]).astype(np.float32) for r in res.results]
    full = np.stack(
        [
            outs[0] + outs[1] + outs[2] + outs[3],
            outs[4] + outs[5] + outs[6] + outs[7],
        ]
    ).astype(np.float32)
    return full, res


def kernel(x, Wq, Wk, Wv, Wo):
    full, _ = _run(x, Wq, Wk, Wv, Wo)
    return full
